# revision 13
# baseline (speedup 1.0000x reference)
"""Multi-head causal attention (kqv proj + softmax(QK^T)V) on 8 TRN2 NeuronCores.

Sharding: 8 cores = 4 batches x 2 head-groups (8 heads each). Each core is
fully independent (no collectives); host shards inputs / concats outputs.

Per-core kernel (bf16 matmuls, f32 psum/output):
  phase 1: Q^T/K^T [64, T] per head (2 heads packed into 128 partitions) and
           V [T, 64] per head produced straight from the kqv matmul -- layouts
           chosen so no on-device transpose is ever needed.
  phase 2: S^T[j,i] = K^T.T @ Q^T per j-tile, both heads of a pair written
           into one 2-bank PSUM tile back-to-back so the second matmul carries
           no semaphore wait and co-executes with the first in the PE array
           (row groups 0/64 via tile_position). Diagonal j-tiles are trimmed
           to the causal column range, in both the S matmul stream and the
           exp. One ScalarE exp per (unit, j-tile) covers both heads, 1/8
           scale fused in, 0/1 mask multiply only on the 128x128 boundary
           block, then out[i,:] = (E^T.T @ [V|1]) normalized by the appended
           denominator column + v-bias.
PSUM budget (8 banks): S ring 2x2 + proj ring 2x1 + pv accumulators 2x1.
Dedicated rings keep the wait-carrying instruction off the S pair's second
matmul, which is what lets the pairs co-execute.

Startup engineering (worth ~8us of the 219us baseline):
  - ~16 dummy matmuls with no DMA dependency run first: the PE DVFS ramp
    (0.65 -> 1.2 -> 2.4 GHz after ~3us of sustained execution) completes
    during the input DMA wait instead of during the projection chains.
  - Input DMAs split across two queues: the first-need wave (xa + pair-0
    weight cols, per-c granularity so the first chain rides the arrival
    wave) issues on the SP HWDGE queue; everything else is merged into
    single big SWDGE DMAs on the idle gpsimd queue (descriptor-gen cost is
    per-instruction, ~650ns on SP -- 74 small DMAs serialized to ~48us of
    issue on the baseline and starved the later waves).
  - PV emits causal-boundary blocks last so the Vector mask-multiply is off
    the PV-start critical path; output DMAs keep SP (issue-cheap there).
"""

import sys

if "/opt/trn_rl_repo" not in sys.path:
    sys.path.insert(0, "/opt/trn_rl_repo")

import numpy as np
import ml_dtypes

DIM = 1024
NUM_HEADS = 16
SEQ = 2048
BATCH = 4
D = 64  # head dim
SCALE = D**-0.5
N_CORES = 8
HPC = 8  # heads per core
PAIRS = HPC // 2
CC = DIM // 128  # contraction chunks (8)
TCH = SEQ // 512  # 512-wide token chunks (4)
TT = SEQ // 128  # 128-wide token tiles (16)
SG = 2  # j-tiles per unit; 4ic+4 is even so units are exact

BF16 = ml_dtypes.bfloat16

_CACHE = {}


def _build_nc():
    import concourse.tile as tile
    from concourse import bacc, mybir

    bf = mybir.dt.bfloat16
    f32 = mybir.dt.float32
    mult = mybir.AluOpType.mult
    add = mybir.AluOpType.add

    nc = bacc.Bacc("TRN2", target_bir_lowering=False)

    # Inputs arrive pre-folded into per-wave tensors whose layout equals the
    # SBUF destination layout, so every wave is ONE DMA with maximal
    # contiguous per-partition lines (the column-sliced variant measured
    # ~116GB/s on 512B lines vs ~330GB/s here).
    xa_d = nc.declare_dram_parameter("xa", [128, CC, 512], bf, isOutput=False)
    xb_d = nc.declare_dram_parameter("xb", [128, CC, 512], bf, isOutput=False)
    xc_d = nc.declare_dram_parameter("xc", [128, CC, 1024], bf, isOutput=False)
    w1k_d = nc.declare_dram_parameter("w1k", [128, CC, 128], bf, isOutput=False)
    w1q_d = nc.declare_dram_parameter("w1q", [128, CC, 128], bf, isOutput=False)
    w2_d = nc.declare_dram_parameter("w2", [128, CC, 256], bf, isOutput=False)
    w3_d = nc.declare_dram_parameter("w3", [128, CC, 512], bf, isOutput=False)
    wv_d = nc.declare_dram_parameter("wv", [128, CC, 512], bf, isOutput=False)
    bqk_d = nc.declare_dram_parameter("bqk", [128, 2 * PAIRS], f32, isOutput=False)
    bv_d = nc.declare_dram_parameter("bv", [128, 512], f32, isOutput=False)
    # output in staged layout; host inverse-permutes. 2KB contiguous lines.
    out_d = nc.declare_dram_parameter(
        "out", [128, TCH, PAIRS, 4, 128], f32, isOutput=True
    )

    with tile.TileContext(nc) as tc:
        with (
            tc.tile_pool(name="persist", bufs=1) as persist,
            tc.tile_pool(name="epool", bufs=16) as epool,
            tc.tile_pool(name="ost", bufs=1) as ost,
            tc.tile_pool(name="rpool", bufs=8) as rpool,
            tc.tile_pool(name="spool", bufs=2, space="PSUM") as spool,
            tc.tile_pool(name="pjpool", bufs=2, space="PSUM") as pjpool,
            tc.tile_pool(name="ppv", bufs=2, space="PSUM") as ppv,
        ):
            # ---- constants ----
            zb = persist.tile([128, 1], f32, tag="zb")
            nc.vector.memset(zb, 0.0)

            # warm-up exp: pulls the one-time ~1.3us ACT_TABLE_LOAD off the
            # first real exp's critical path (runs during the DMA wait)
            warm = persist.tile([128, 1], f32, tag="warm")
            nc.scalar.activation(
                warm, zb, mybir.ActivationFunctionType.Exp, bias=zb, scale=1.0
            )

            # PE clock warm-up: the DVFS ramp needs ~3us of sustained matmul
            # execution to reach 2.4 GHz. These dummies depend only on a
            # memset, so they run during the initial DMA wait; without them
            # the DMA-paced projection chains hold the PE at 1.2 GHz for the
            # first ~15us. dst is the first ppv ring slot, retired before
            # its first real use.
            wsrc = persist.tile([128, 260], bf, tag="wsrc")
            nc.vector.memset(wsrc, 0.0)
            pewarm = ppv.tile([128, 4, D + 1], f32, tag="pv", name="pewarm")
            for _ in range(16):
                nc.tensor.matmul(
                    pewarm[:].rearrange("p a b -> p (a b)"),
                    wsrc[:, 0:128],
                    wsrc[:],
                )

            # ---- inputs ----
            # All input DMAs go through the ONE SP HWDGE queue, merged into
            # few big instructions (descriptor gen is ~650ns per instruction
            # regardless of size -- 74 small DMAs serialized to ~48us of
            # issue on the original baseline), ordered strictly by first
            # need. A single queue self-paces: transfers arrive in exactly
            # this order at full bus bandwidth. (A two-queue variant was
            # measured WORSE: the second queue's early flood split the
            # ~358GB/s bus and starved the critical first wave.)
            # Per-wave SBUF tiles mirror the pre-folded dram layout exactly,
            # so every wave DMA is fully contiguous on BOTH sides (one
            # 2-8KB descriptor per partition; SBUF-side column-slicing into
            # a combined tile measured 2x slower from <512B runs).
            xa = persist.tile([128, CC, 512], bf, tag="xa", name="xa")
            xb = persist.tile([128, CC, 512], bf, tag="xb", name="xb")
            xc = persist.tile([128, CC, 1024], bf, tag="xc", name="xc")
            w1k = persist.tile([128, CC, 128], bf, tag="w1k", name="w1k")
            w1q = persist.tile([128, CC, 128], bf, tag="w1q", name="w1q")
            w2 = persist.tile([128, CC, 256], bf, tag="w2", name="w2")
            w3 = persist.tile([128, CC, 512], bf, tag="w3", name="w3")
            wv = persist.tile([128, CC, 512], bf, tag="wv", name="wv")
            bqk_sb = persist.tile([128, 2 * PAIRS], f32, tag="bqk")
            bv_sb = persist.tile([128, 512], f32, tag="bv")

            def w_cols(c, p, which):
                """pair p's k (which=1) / q (which=0) weight block, chunk c"""
                if p == 0:
                    return (w1k if which else w1q)[:, c, :]
                if p == 1:
                    off = 0 if which else 128
                    return w2[:, c, off : off + 128]
                off = (p - 2) * 256 + (0 if which else 128)
                return w3[:, c, off : off + 128]

            # first projection chains ride this arrival sequence: pair-0 k
            # cols first (the k chain is queued first for chunk (0,0)), then
            # xa piecewise (the chain's c-th matmul only needs the piece
            # holding chunk c), q cols, biases, later waves.
            nc.sync.dma_start(out=w1k[:], in_=w1k_d[:])
            nc.sync.dma_start(out=xa[:, 0:2, :], in_=xa_d[:, 0:2, :])
            nc.sync.dma_start(out=w1q[:], in_=w1q_d[:])
            nc.sync.dma_start(out=xa[:, 2:4, :], in_=xa_d[:, 2:4, :])
            nc.sync.dma_start(out=xa[:, 4:6, :], in_=xa_d[:, 4:6, :])
            nc.sync.dma_start(out=xa[:, 6:8, :], in_=xa_d[:, 6:8, :])
            nc.sync.dma_start(out=bqk_sb, in_=bqk_d[:])
            nc.sync.dma_start(out=bv_sb, in_=bv_d[:])
            nc.sync.dma_start(out=wv[:], in_=wv_d[:])
            nc.sync.dma_start(out=w2[:], in_=w2_d[:])
            nc.sync.dma_start(out=xb[:], in_=xb_d[:])
            nc.sync.dma_start(out=w3[:], in_=w3_d[:])
            nc.sync.dma_start(out=xc[:], in_=xc_d[:])

            # single causal boundary mask: mask[jj, ii] = 1 if ii >= jj
            mask = persist.tile([128, 128], bf, tag="mask")
            nc.gpsimd.memset(mask, 1.0)
            nc.gpsimd.affine_select(
                out=mask,
                in_=mask,
                compare_op=mybir.AluOpType.is_ge,
                fill=0.0,
                base=0,
                pattern=[[1, 128]],
                channel_multiplier=-1,
            )

            def x_cols(c, lo, width):
                """view of xT[c][:, lo:lo+width] from the wave tiles"""
                if lo + width <= 512:
                    return xa[:, c, lo : lo + width]
                if lo >= 512 and lo + width <= 1024:
                    return xb[:, c, lo - 512 : lo - 512 + width]
                assert lo >= 1024
                return xc[:, c, lo - 1024 : lo - 1024 + width]

            QT = [persist.tile([128, SEQ], bf, tag=f"qt{p}", name=f"qt{p}") for p in range(PAIRS)]
            KT = [persist.tile([128, SEQ], bf, tag=f"kt{p}", name=f"kt{p}") for p in range(PAIRS)]
            Vp = [
                persist.tile([128, HPC, D + 1], bf, tag=f"vp{t}", name=f"vp{t}")
                for t in range(TT)
            ]

            def proj_qk_chunk(p, which, t):
                # which: 0=q, 1=k. Host layout: [k p | q p] per pair.
                dst = QT[p] if which == 0 else KT[p]
                bcol = p if which == 0 else PAIRS + p
                ps = pjpool.tile([128, 512], f32, tag="pj", name=f"pqk{p}_{which}_{t}")
                for c in range(CC):
                    nc.tensor.matmul(
                        ps[:],
                        w_cols(c, p, which),
                        x_cols(c, t * 512, 512),
                        start=(c == 0),
                        stop=(c == CC - 1),
                    )
                nc.vector.tensor_scalar_add(
                    dst[:, t * 512 : (t + 1) * 512],
                    ps[:],
                    bqk_sb[:, bcol : bcol + 1],
                )

            def proj_v(tt):
                ps = pjpool.tile([128, 512], f32, tag="pj", name=f"pv{tt}")
                for c in range(CC):
                    nc.tensor.matmul(
                        ps[:],
                        x_cols(c, tt * 128, 128),
                        wv[:, c, :],
                        start=(c == 0),
                        stop=(c == CC - 1),
                    )
                nc.vector.tensor_copy(
                    out=Vp[tt][:, :, 0:D],
                    in_=ps[:].rearrange("p (h d) -> p h d", h=HPC),
                )
                nc.vector.memset(Vp[tt][:, :, D : D + 1], 1.0)

            # ---- attention, software-pipelined at S-unit granularity ----
            # Unit = (pr, ic, tiles): 1-2 j-tiles x 2 heads. Emission order is
            # S+exp(unit k+1) then PV(unit k), so the PE computes the next
            # unit's scores while ScalarE exps. Per j-tile, the two heads'
            # matmuls go back-to-back into one 2-bank psum tile from a
            # dedicated ring so the pair co-executes; the exp for that j-tile
            # is issued immediately so ScalarE starts while the next pair
            # streams. Diagonal tiles stream/exp only cols >= 128*r (the
            # columns below are never read: PV skips those i-blocks).
            pvs = {}

            def emit_s_exp(pr, ic, tiles):
                es = []
                for jt in tiles:
                    r = jt - 4 * ic
                    lo = 128 * r if r > 0 else 0
                    ps = spool.tile(
                        [128, 2, 512], f32, tag="s", name=f"s_{pr}_{ic}_{jt}"
                    )
                    for half in (0, 1):
                        po = half * D
                        nc.tensor.matmul(
                            ps[:, half, lo:],
                            KT[pr][po : po + D, jt * 128 : (jt + 1) * 128],
                            QT[pr][po : po + D, ic * 512 + lo : (ic + 1) * 512],
                            tile_position=(po, 0),
                        )
                    e = epool.tile([128, 2, 512], bf, tag="e")
                    if lo:
                        exp_dst, exp_src = e[:, :, lo:], ps[:, :, lo:]
                    else:
                        exp_dst = e[:].rearrange("p a b -> p (a b)")
                        exp_src = ps[:].rearrange("p a b -> p (a b)")
                    nc.scalar.activation(
                        exp_dst,
                        exp_src,
                        mybir.ActivationFunctionType.Exp,
                        bias=zb,
                        scale=SCALE,
                    )
                    if r >= 0:  # diagonal tile: mask the boundary block
                        # one multiply covers both heads' blocks (mask
                        # broadcast along the half dim)
                        blk = e[:, :, r * 128 : (r + 1) * 128]
                        nc.vector.tensor_tensor(
                            blk,
                            blk,
                            mask[:, None, :].to_broadcast([128, 2, 128]),
                            mult,
                        )
                    es.append(e)
                return (pr, ic, tiles, es)

            def emit_pv(unit):
                pr, ic, tiles, es = unit
                if tiles[0] == 0:
                    for hh in (0, 1):
                        pvs[hh] = ppv.tile(
                            [128, 4, D + 1], f32, tag="pv", name=f"pv_{pr}_{ic}_{hh}"
                        )
                # has_written is cleared bank-wide by start=True, so only the
                # first matmul into the pv tile may carry start=True;
                # start=False matmuls overwrite where the bit is unset, which
                # correctly begins the other three chains.
                # The causal-boundary block (itl == jt-4ic) is emitted LAST:
                # it depends on the Vector mask-multiply, the others only on
                # the exp, so PV starts without waiting for Vector.
                for k, jt in enumerate(tiles):
                    e = es[k]
                    r = jt - 4 * ic
                    if r < 0:
                        itls = list(range(4))
                    else:
                        itls = list(range(r + 1, 4)) + [r]
                    for half in (0, 1):
                        h = 2 * pr + half
                        for itl in itls:
                            nc.tensor.matmul(
                                pvs[half][:, itl, :],
                                e[:, half, itl * 128 : (itl + 1) * 128],
                                Vp[jt][:, h, :],
                                start=(jt == 0 and itl == itls[0]),
                                stop=(jt == 4 * ic + 3 and itl == r),
                            )

            def maybe_epilogue(unit):
                pr, ic, tiles, _ = unit
                if tiles[-1] + 1 != 4 * ic + 4:
                    return
                stage = stages[ic]
                for half in (0, 1):
                    h = 2 * pr + half
                    rec = rpool.tile([128, 4], f32, tag="rec")
                    nc.vector.reciprocal(rec, pvs[half][:, :, D])
                    seg = stage[:, :, h * D : (h + 1) * D]
                    nc.vector.tensor_tensor(
                        seg,
                        pvs[half][:, :, 0:D],
                        rec[:, :, None].to_broadcast([128, 4, D]),
                        mult,
                    )
                    nc.vector.tensor_tensor(
                        seg,
                        seg,
                        bv_sb[:, None, h * D : (h + 1) * D].to_broadcast([128, 4, D]),
                        add,
                    )
                # stream this pair's 128-col output segment immediately
                nc.sync.dma_start(
                    out=out_d[:, ic, pr, :, :],
                    in_=stage[:, :, pr * 128 : (pr + 1) * 128],
                )

            stages = [
                ost.tile([128, 4, 512], f32, tag=f"ostage{ic}", name=f"stage_{ic}")
                for ic in range(TCH)
            ]

            # Chunks run in wavefront order -- (0,0),(1,0),(0,1),(2,0),... --
            # so ACT-heavy later-pair chunks overlap the projection-heavy
            # early window. NOTE a chunk (pr,ic) consumes KT[pr] cols
            # 0:(4ic+4)*128, i.e. the k-chains of ALL (pr,ic'<=ic) -- chunks
            # cannot be reordered past a lower-ic chunk of the same pair.
            # Projection chains are due at the unit index that first needs
            # them: mandatory drain at their due unit, plus one lookahead pop
            # per unit to smooth PE load.
            chunks = sorted(
                [(pr, ic) for pr in range(PAIRS) for ic in range(TCH)],
                key=lambda c: (c[0] + c[1], -c[0]),
            )
            units = []
            for ci, (pr, ic) in enumerate(chunks):
                tl = list(range(4 * ic + 4))
                if ci == len(chunks) - 1:
                    # last chunk: final two j-tiles run as single-tile units
                    # so the terminal pipeline drain is one j-tile shorter
                    groups = [tuple(tl[g : g + SG]) for g in range(0, 4 * ic + 2, SG)]
                    groups += [(tl[-2],), (tl[-1],)]
                else:
                    groups = [tuple(tl[g : g + SG]) for g in range(0, 4 * ic + 4, SG)]
                for g in groups:
                    units.append((pr, ic, g))
            uidx = {}
            for i, (pr, ic, g) in enumerate(units):
                for jt in g:
                    uidx[(pr, ic, jt)] = i

            queue = []
            for pr, ic in chunks:
                due = uidx[(pr, ic, 0)]
                # q is needed by the chunk's FIRST unit, but this chunk's own
                # k-cols (KT[:, ic*512:(ic+1)*512]) are only read by the
                # DIAGONAL j-tiles -- the last two units. Deferring the
                # k-chain shrinks the chunk-start proj burst from 3.4us to
                # 1.7us, so exp issue (1:1 on the span) resumes sooner.
                kdue = max(due, uidx[(pr, ic, 4 * ic)] - 1)
                # k before q when both are due together (chunk (0,0)): the k
                # weight cols are first in the DMA arrival order
                queue.append((kdue, lambda pr=pr, ic=ic: proj_qk_chunk(pr, 1, ic)))
                queue.append((due, lambda pr=pr, ic=ic: proj_qk_chunk(pr, 0, ic)))
            for t in range(TT):
                # PV of unit u is emitted during iteration u+1 (after that
                # iteration's drain), so V tiles are due one unit late --
                # keeps V chains off the first-exp critical path. First-need
                # is the earliest unit (any pair) whose j-tiles cover t.
                due_v = (
                    min(
                        uidx[(pr, ic, t)]
                        for pr in range(PAIRS)
                        for ic in range(TCH)
                        if 4 * ic + 4 > t
                    )
                    + 1
                )
                queue.append((due_v, lambda t=t: proj_v(t)))
            queue.sort(key=lambda kv: kv[0])

            qi = [0]

            def drain_due(i, lookahead=0, limit=None):
                n = 0
                while qi[0] < len(queue) and queue[qi[0]][0] <= i + lookahead:
                    if lookahead and limit is not None and n >= limit:
                        break
                    queue[qi[0]][1]()
                    qi[0] += 1
                    n += 1

            # S(u) strictly before PV(u-1): exp(u) must issue as early as
            # possible -- ScalarE is lockstep-coupled and any delay in exp
            # issue extends the span ~1:1 (PV-first orderings measured
            # consistently worse).
            pend = []
            for i, (pr, ic, g) in enumerate(units):
                drain_due(i)
                pend.append(emit_s_exp(pr, ic, g))
                if len(pend) > 1:
                    u = pend.pop(0)
                    emit_pv(u)
                    maybe_epilogue(u)
                drain_due(i, lookahead=4, limit=1)
            for u in pend:
                emit_pv(u)
                maybe_epilogue(u)
            drain_due(len(units))

    nc.compile()
    return nc


def _get_nc():
    if "nc" not in _CACHE:
        _CACHE["nc"] = _build_nc()
    return _CACHE["nc"]


def _fold(arr):
    """[DIM, cols] -> [128, CC, cols] partition-folded wave layout."""
    return np.ascontiguousarray(
        arr.reshape(CC, 128, arr.shape[1]).transpose(1, 0, 2)
    ).astype(BF16)


def _make_in_maps(x, w_kqv, b_kqv):
    """Shard: core c -> batch c//2, head-group c%2 (heads hg*8..hg*8+7)."""
    in_maps = []
    for c in range(N_CORES):
        b, hg = divmod(c, 2)
        h0 = hg * HPC
        xT = x[b].T

        # weight columns, all transposed to [DIM(c), out]:
        # [k p0 | q p0 | k p1 | q p1 | ... | v heads (512)]
        # (k rows live at 0+, q rows at 1024+ in the torch kqv weight)
        cols = []
        for p in range(PAIRS):
            for which in (1, 0):
                base = (1 - which) * DIM  # which==1 -> k rows, 0 -> q rows
                rows = w_kqv[base + (h0 + 2 * p) * D : base + (h0 + 2 * p + 2) * D, :]
                cols.append(rows.T)
        cols.append(w_kqv[2 * DIM + h0 * D : 2 * DIM + (h0 + HPC) * D, :].T)
        wT = np.concatenate(cols, axis=1)

        bqk = np.empty((128, 2 * PAIRS), np.float32)
        for p in range(PAIRS):
            bqk[:, p] = b_kqv[DIM + (h0 + 2 * p) * D : DIM + (h0 + 2 * p + 2) * D]
            bqk[:, PAIRS + p] = b_kqv[(h0 + 2 * p) * D : (h0 + 2 * p + 2) * D]
        bv = np.tile(
            b_kqv[2 * DIM + h0 * D : 2 * DIM + (h0 + HPC) * D][None, :].astype(
                np.float32
            ),
            (128, 1),
        )
        in_maps.append(
            {
                "xa": _fold(xT[:, 0:512]),
                "xb": _fold(xT[:, 512:1024]),
                "xc": _fold(xT[:, 1024:2048]),
                "w1k": _fold(wT[:, 0:128]),
                "w1q": _fold(wT[:, 128:256]),
                "w2": _fold(wT[:, 256:512]),
                "w3": _fold(wT[:, 512:1024]),
                "wv": _fold(wT[:, 1024:1536]),
                "bqk": bqk,
                "bv": bv,
            }
        )
    return in_maps


def run(x, w_kqv, b_kqv, trace=False, **kwargs):
    from concourse.bass_utils import run_bass_kernel_spmd

    nc = _get_nc()
    in_maps = _make_in_maps(x, w_kqv, b_kqv)
    res = run_bass_kernel_spmd(
        nc, in_maps, core_ids=list(range(N_CORES)), trace=trace, **kwargs
    )
    out = np.empty((BATCH, SEQ, DIM), np.float32)
    for c in range(N_CORES):
        b, hg = divmod(c, 2)
        # staged layout [p, ic, pr, a, c] -> [ic*512 + a*128 + p, pr*128 + c]
        arr = res.results[c]["out"]
        out[b, :, hg * 512 : (hg + 1) * 512] = arr.transpose(1, 3, 0, 2, 4).reshape(
            SEQ, 512
        )
    return out, res


def kernel(x, w_kqv, b_kqv):
    args = (
        np.asarray(x, np.float32),
        np.asarray(w_kqv, np.float32),
        np.asarray(b_kqv, np.float32),
    )
    try:
        out, _ = run(*args)
    except Exception:
        # transient NRT/device errors recover on retry
        out, _ = run(*args)
    return out


# revision 17
# speedup vs baseline: 1.1815x; 1.1815x over previous
"""Multi-head causal attention (kqv proj + softmax(QK^T)V) on 8 TRN2 NeuronCores.

Sharding: 8 cores = 4 batches x 2 head-groups (8 heads each). Each core is
fully independent (no collectives); host shards inputs / concats outputs.

Per-core kernel (bf16 matmuls, f32 psum/output):
  phase 1: Q^T/K^T [64, T] per head (2 heads packed into 128 partitions) and
           V [T, 64] per head produced straight from the kqv matmul -- layouts
           chosen so no on-device transpose is ever needed.
  phase 2: S^T[j,i] = K^T.T @ Q^T per j-tile, both heads of a pair written
           into one 2-bank PSUM tile back-to-back so the second matmul carries
           no semaphore wait and co-executes with the first in the PE array
           (row groups 0/64 via tile_position). Diagonal j-tiles are trimmed
           to the causal column range, in both the S matmul stream and the
           exp. One ScalarE exp per (unit, j-tile) covers both heads, 1/8
           scale fused in, 0/1 mask multiply only on the 128x128 boundary
           block, then out[i,:] = (E^T.T @ [V|1]) normalized by the appended
           denominator column + v-bias.
PSUM budget (8 banks): S ring 2x2 + proj ring 2x1 + pv accumulators 2x1.
Dedicated rings keep the wait-carrying instruction off the S pair's second
matmul, which is what lets the pairs co-execute.

Startup engineering (worth ~8us of the 219us baseline):
  - ~16 dummy matmuls with no DMA dependency run first: the PE DVFS ramp
    (0.65 -> 1.2 -> 2.4 GHz after ~3us of sustained execution) completes
    during the input DMA wait instead of during the projection chains.
  - Input DMAs split across two queues: the first-need wave (xa + pair-0
    weight cols, per-c granularity so the first chain rides the arrival
    wave) issues on the SP HWDGE queue; everything else is merged into
    single big SWDGE DMAs on the idle gpsimd queue (descriptor-gen cost is
    per-instruction, ~650ns on SP -- 74 small DMAs serialized to ~48us of
    issue on the baseline and starved the later waves).
  - PV emits causal-boundary blocks last so the Vector mask-multiply is off
    the PV-start critical path; output DMAs keep SP (issue-cheap there).
"""

import sys

if "/opt/trn_rl_repo" not in sys.path:
    sys.path.insert(0, "/opt/trn_rl_repo")

import numpy as np
import ml_dtypes

DIM = 1024
NUM_HEADS = 16
SEQ = 2048
BATCH = 4
D = 64  # head dim
SCALE = D**-0.5
N_CORES = 8
HPC = 8  # heads per core
PAIRS = HPC // 2
CC = DIM // 128  # contraction chunks (8)
TCH = SEQ // 512  # 512-wide token chunks (4)
TT = SEQ // 128  # 128-wide token tiles (16)
SG = 2  # j-tiles per unit; 4ic+4 is even so units are exact

BF16 = ml_dtypes.bfloat16

_CACHE = {}


def _build_nc():
    import concourse.tile as tile
    from concourse import bacc, mybir

    bf = mybir.dt.bfloat16
    f32 = mybir.dt.float32
    mult = mybir.AluOpType.mult
    add = mybir.AluOpType.add

    nc = bacc.Bacc("TRN2", target_bir_lowering=False)

    # Inputs arrive pre-folded into per-wave tensors whose layout equals the
    # SBUF destination layout, so every wave is ONE DMA with maximal
    # contiguous per-partition lines (the column-sliced variant measured
    # ~116GB/s on 512B lines vs ~330GB/s here).
    xa_d = nc.declare_dram_parameter("xa", [128, CC, 512], bf, isOutput=False)
    xb_d = nc.declare_dram_parameter("xb", [128, CC, 512], bf, isOutput=False)
    xc_d = nc.declare_dram_parameter("xc", [128, CC, 1024], bf, isOutput=False)
    w1k_d = nc.declare_dram_parameter("w1k", [128, CC, 128], bf, isOutput=False)
    w1q_d = nc.declare_dram_parameter("w1q", [128, CC, 128], bf, isOutput=False)
    w2_d = nc.declare_dram_parameter("w2", [128, CC, 256], bf, isOutput=False)
    w3_d = nc.declare_dram_parameter("w3", [128, CC, 512], bf, isOutput=False)
    wv_d = nc.declare_dram_parameter("wv", [128, CC, 512], bf, isOutput=False)
    bqk_d = nc.declare_dram_parameter("bqk", [128, 2 * PAIRS], f32, isOutput=False)
    bv_d = nc.declare_dram_parameter("bv", [128, 512], f32, isOutput=False)
    # output in staged layout; host inverse-permutes. 2KB contiguous lines.
    out_d = nc.declare_dram_parameter(
        "out", [128, TCH, PAIRS, 4, 128], f32, isOutput=True
    )

    with tile.TileContext(nc) as tc:
        with (
            tc.tile_pool(name="persist", bufs=1) as persist,
            tc.tile_pool(name="epool", bufs=16) as epool,
            tc.tile_pool(name="ost", bufs=1) as ost,
            tc.tile_pool(name="rpool", bufs=8) as rpool,
            tc.tile_pool(name="spool", bufs=2, space="PSUM") as spool,
            tc.tile_pool(name="pjpool", bufs=2, space="PSUM") as pjpool,
            tc.tile_pool(name="ppv", bufs=2, space="PSUM") as ppv,
        ):
            # ---- constants ----
            zb = persist.tile([128, 1], f32, tag="zb")
            nc.vector.memset(zb, 0.0)

            # warm-up exp: pulls the one-time ~1.3us ACT_TABLE_LOAD off the
            # first real exp's critical path (runs during the DMA wait)
            warm = persist.tile([128, 1], f32, tag="warm")
            nc.scalar.activation(
                warm, zb, mybir.ActivationFunctionType.Exp, bias=zb, scale=1.0
            )

            # PE clock warm-up: the DVFS ramp needs ~3us of sustained matmul
            # execution to reach 2.4 GHz. These dummies depend only on a
            # memset, so they run during the initial DMA wait; without them
            # the DMA-paced projection chains hold the PE at 1.2 GHz for the
            # first ~15us. dst is the first ppv ring slot, retired before
            # its first real use.
            wsrc = persist.tile([128, 260], bf, tag="wsrc")
            nc.vector.memset(wsrc, 0.0)
            pewarm = ppv.tile([128, 4, D + 1], f32, tag="pv", name="pewarm")
            for _ in range(16):
                nc.tensor.matmul(
                    pewarm[:].rearrange("p a b -> p (a b)"),
                    wsrc[:, 0:128],
                    wsrc[:],
                )

            # ---- inputs ----
            # All input DMAs go through the ONE SP HWDGE queue, merged into
            # few big instructions (descriptor gen is ~650ns per instruction
            # regardless of size -- 74 small DMAs serialized to ~48us of
            # issue on the original baseline), ordered strictly by first
            # need. A single queue self-paces: transfers arrive in exactly
            # this order at full bus bandwidth. (A two-queue variant was
            # measured WORSE: the second queue's early flood split the
            # ~358GB/s bus and starved the critical first wave.)
            # Per-wave SBUF tiles mirror the pre-folded dram layout exactly,
            # so every wave DMA is fully contiguous on BOTH sides (one
            # 2-8KB descriptor per partition; SBUF-side column-slicing into
            # a combined tile measured 2x slower from <512B runs).
            xa = persist.tile([128, CC, 512], bf, tag="xa", name="xa")
            xb = persist.tile([128, CC, 512], bf, tag="xb", name="xb")
            xc = persist.tile([128, CC, 1024], bf, tag="xc", name="xc")
            w1k = persist.tile([128, CC, 128], bf, tag="w1k", name="w1k")
            w1q = persist.tile([128, CC, 128], bf, tag="w1q", name="w1q")
            w2 = persist.tile([128, CC, 256], bf, tag="w2", name="w2")
            w3 = persist.tile([128, CC, 512], bf, tag="w3", name="w3")
            wv = persist.tile([128, CC, 512], bf, tag="wv", name="wv")
            bqk_sb = persist.tile([128, 2 * PAIRS], f32, tag="bqk")
            bv_sb = persist.tile([128, 512], f32, tag="bv")

            def w_cols(c, p, which):
                """pair p's k (which=1) / q (which=0) weight block, chunk c"""
                if p == 0:
                    return (w1k if which else w1q)[:, c, :]
                if p == 1:
                    off = 0 if which else 128
                    return w2[:, c, off : off + 128]
                off = (p - 2) * 256 + (0 if which else 128)
                return w3[:, c, off : off + 128]

            # first projection chains ride this arrival sequence: pair-0 k
            # cols first (the k chain is queued first for chunk (0,0)), then
            # xa piecewise (the chain's c-th matmul only needs the piece
            # holding chunk c), q cols, biases, later waves.
            # tiny bias DMAs ride the idle gpsimd SWDGE queue so they don't
            # spend SP issue slots ahead of the xa waves
            nc.gpsimd.dma_start(out=bqk_sb, in_=bqk_d[:])
            nc.gpsimd.dma_start(out=bv_sb, in_=bv_d[:])
            nc.sync.dma_start(out=w1k[:], in_=w1k_d[:])
            nc.sync.dma_start(out=xa[:, 0:2, :], in_=xa_d[:, 0:2, :])
            nc.sync.dma_start(out=w1q[:], in_=w1q_d[:])
            nc.sync.dma_start(out=xa[:, 2:4, :], in_=xa_d[:, 2:4, :])
            nc.sync.dma_start(out=xa[:, 4:6, :], in_=xa_d[:, 4:6, :])
            nc.sync.dma_start(out=xa[:, 6:8, :], in_=xa_d[:, 6:8, :])
            nc.sync.dma_start(out=wv[:], in_=wv_d[:])
            nc.sync.dma_start(out=w2[:], in_=w2_d[:])
            nc.sync.dma_start(out=xb[:], in_=xb_d[:])
            nc.sync.dma_start(out=w3[:], in_=w3_d[:])
            nc.sync.dma_start(out=xc[:], in_=xc_d[:])

            # single causal boundary mask: mask[jj, ii] = 1 if ii >= jj
            mask = persist.tile([128, 128], bf, tag="mask")
            nc.gpsimd.memset(mask, 1.0)
            nc.gpsimd.affine_select(
                out=mask,
                in_=mask,
                compare_op=mybir.AluOpType.is_ge,
                fill=0.0,
                base=0,
                pattern=[[1, 128]],
                channel_multiplier=-1,
            )

            def x_cols(c, lo, width):
                """view of xT[c][:, lo:lo+width] from the wave tiles"""
                if lo + width <= 512:
                    return xa[:, c, lo : lo + width]
                if lo >= 512 and lo + width <= 1024:
                    return xb[:, c, lo - 512 : lo - 512 + width]
                assert lo >= 1024
                return xc[:, c, lo - 1024 : lo - 1024 + width]

            QT = [persist.tile([128, SEQ], bf, tag=f"qt{p}", name=f"qt{p}") for p in range(PAIRS)]
            KT = [persist.tile([128, SEQ], bf, tag=f"kt{p}", name=f"kt{p}") for p in range(PAIRS)]
            Vp = [
                persist.tile([128, HPC, D + 1], bf, tag=f"vp{t}", name=f"vp{t}")
                for t in range(TT)
            ]

            def proj_qk_chunk(p, which, t):
                # which: 0=q, 1=k. Host layout: [k p | q p] per pair.
                dst = QT[p] if which == 0 else KT[p]
                bcol = p if which == 0 else PAIRS + p
                ps = pjpool.tile([128, 512], f32, tag="pj", name=f"pqk{p}_{which}_{t}")
                for c in range(CC):
                    nc.tensor.matmul(
                        ps[:],
                        w_cols(c, p, which),
                        x_cols(c, t * 512, 512),
                        start=(c == 0),
                        stop=(c == CC - 1),
                    )
                nc.vector.tensor_scalar_add(
                    dst[:, t * 512 : (t + 1) * 512],
                    ps[:],
                    bqk_sb[:, bcol : bcol + 1],
                )

            def proj_v(tt):
                ps = pjpool.tile([128, 512], f32, tag="pj", name=f"pv{tt}")
                for c in range(CC):
                    nc.tensor.matmul(
                        ps[:],
                        x_cols(c, tt * 128, 128),
                        wv[:, c, :],
                        start=(c == 0),
                        stop=(c == CC - 1),
                    )
                nc.vector.tensor_copy(
                    out=Vp[tt][:, :, 0:D],
                    in_=ps[:].rearrange("p (h d) -> p h d", h=HPC),
                )
                nc.vector.memset(Vp[tt][:, :, D : D + 1], 1.0)

            # ---- attention, software-pipelined at S-unit granularity ----
            # Unit = (pr, ic, tiles): 1-2 j-tiles x 2 heads. Emission order is
            # S+exp(unit k+1) then PV(unit k), so the PE computes the next
            # unit's scores while ScalarE exps. Per j-tile, the two heads'
            # matmuls go back-to-back into one 2-bank psum tile from a
            # dedicated ring so the pair co-executes; the exp for that j-tile
            # is issued immediately so ScalarE starts while the next pair
            # streams. Diagonal tiles stream/exp only cols >= 128*r (the
            # columns below are never read: PV skips those i-blocks).
            pvs = {}

            def emit_s_exp(pr, ic, tiles):
                es = []
                for jt in tiles:
                    r = jt - 4 * ic
                    lo = 128 * r if r > 0 else 0
                    ps = spool.tile(
                        [128, 2, 512], f32, tag="s", name=f"s_{pr}_{ic}_{jt}"
                    )
                    for half in (0, 1):
                        po = half * D
                        nc.tensor.matmul(
                            ps[:, half, lo:],
                            KT[pr][po : po + D, jt * 128 : (jt + 1) * 128],
                            QT[pr][po : po + D, ic * 512 + lo : (ic + 1) * 512],
                            tile_position=(po, 0),
                        )
                    e = epool.tile([128, 2, 512], bf, tag="e")
                    if lo:
                        exp_dst, exp_src = e[:, :, lo:], ps[:, :, lo:]
                    else:
                        exp_dst = e[:].rearrange("p a b -> p (a b)")
                        exp_src = ps[:].rearrange("p a b -> p (a b)")
                    nc.scalar.activation(
                        exp_dst,
                        exp_src,
                        mybir.ActivationFunctionType.Exp,
                        bias=zb,
                        scale=SCALE,
                    )
                    if r >= 0:  # diagonal tile: mask the boundary block
                        # one multiply covers both heads' blocks (mask
                        # broadcast along the half dim)
                        blk = e[:, :, r * 128 : (r + 1) * 128]
                        nc.vector.tensor_tensor(
                            blk,
                            blk,
                            mask[:, None, :].to_broadcast([128, 2, 128]),
                            mult,
                        )
                    es.append(e)
                return (pr, ic, tiles, es)

            def emit_pv(unit):
                pr, ic, tiles, es = unit
                if tiles[0] == 0:
                    for hh in (0, 1):
                        pvs[hh] = ppv.tile(
                            [128, 4, D + 1], f32, tag="pv", name=f"pv_{pr}_{ic}_{hh}"
                        )
                # has_written is cleared bank-wide by start=True, so only the
                # first matmul into the pv tile may carry start=True;
                # start=False matmuls overwrite where the bit is unset, which
                # correctly begins the other three chains.
                # The causal-boundary block (itl == jt-4ic) is emitted LAST:
                # it depends on the Vector mask-multiply, the others only on
                # the exp, so PV starts without waiting for Vector.
                for k, jt in enumerate(tiles):
                    e = es[k]
                    r = jt - 4 * ic
                    if r < 0:
                        itls = list(range(4))
                    else:
                        itls = list(range(r + 1, 4)) + [r]
                    for half in (0, 1):
                        h = 2 * pr + half
                        for itl in itls:
                            # per-region stop: region itl's chain ends at its
                            # causal-boundary tile jt == 4ic+itl, which lets
                            # the staged epilogue read finished regions while
                            # later regions still accumulate
                            nc.tensor.matmul(
                                pvs[half][:, itl, :],
                                e[:, half, itl * 128 : (itl + 1) * 128],
                                Vp[jt][:, h, :],
                                start=(jt == 0 and itl == itls[0]),
                                stop=(jt == 4 * ic + itl),
                            )

            def emit_epilogue(pr, ic, a0, a1):
                """normalize + bias + stream out rows itl a0..a1 of the pair"""
                stage = stages[ic]
                n = a1 - a0
                for half in (0, 1):
                    h = 2 * pr + half
                    rec = rpool.tile([128, 4], f32, tag="rec")
                    nc.vector.reciprocal(rec[:, 0:n], pvs[half][:, a0:a1, D])
                    seg = stage[:, a0:a1, h * D : (h + 1) * D]
                    nc.vector.tensor_tensor(
                        seg,
                        pvs[half][:, a0:a1, 0:D],
                        rec[:, 0:n, None].to_broadcast([128, n, D]),
                        mult,
                    )
                    nc.vector.tensor_tensor(
                        seg,
                        seg,
                        bv_sb[:, None, h * D : (h + 1) * D].to_broadcast([128, n, D]),
                        add,
                    )
                nc.sync.dma_start(
                    out=out_d[:, ic, pr, a0:a1, :],
                    in_=stage[:, a0:a1, pr * 128 : (pr + 1) * 128],
                )

            def maybe_epilogue(unit, staged):
                pr, ic, tiles, _ = unit
                if staged:
                    # final chunk: epilogue per finished pv region, so the
                    # terminal pipeline drain only carries the last region's
                    # normalize + one small DMA
                    rs = [jt - 4 * ic for jt in tiles if jt >= 4 * ic]
                    if rs:
                        emit_epilogue(pr, ic, rs[0], rs[-1] + 1)
                elif tiles[-1] + 1 == 4 * ic + 4:
                    emit_epilogue(pr, ic, 0, 4)

            stages = [
                ost.tile([128, 4, 512], f32, tag=f"ostage{ic}", name=f"stage_{ic}")
                for ic in range(TCH)
            ]

            # Chunks run in wavefront order -- (0,0),(1,0),(0,1),(2,0),... --
            # so ACT-heavy later-pair chunks overlap the projection-heavy
            # early window. NOTE a chunk (pr,ic) consumes KT[pr] cols
            # 0:(4ic+4)*128, i.e. the k-chains of ALL (pr,ic'<=ic) -- chunks
            # cannot be reordered past a lower-ic chunk of the same pair.
            # Projection chains are due at the unit index that first needs
            # them: mandatory drain at their due unit, plus one lookahead pop
            # per unit to smooth PE load.
            chunks = sorted(
                [(pr, ic) for pr in range(PAIRS) for ic in range(TCH)],
                key=lambda c: (c[0] + c[1], -c[0]),
            )
            units = []
            for ci, (pr, ic) in enumerate(chunks):
                tl = list(range(4 * ic + 4))
                if ci == len(chunks) - 1:
                    # last chunk: final two j-tiles run as single-tile units
                    # so the terminal pipeline drain is one j-tile shorter
                    groups = [tuple(tl[g : g + SG]) for g in range(0, 4 * ic + 2, SG)]
                    groups += [(tl[-2],), (tl[-1],)]
                else:
                    groups = [tuple(tl[g : g + SG]) for g in range(0, 4 * ic + 4, SG)]
                for g in groups:
                    units.append((pr, ic, g))
            uidx = {}
            for i, (pr, ic, g) in enumerate(units):
                for jt in g:
                    uidx[(pr, ic, jt)] = i

            queue = []
            for pr, ic in chunks:
                due = uidx[(pr, ic, 0)]
                # q is needed by the chunk's FIRST unit, but this chunk's own
                # k-cols (KT[:, ic*512:(ic+1)*512]) are only read by the
                # DIAGONAL j-tiles -- the last two units. Deferring the
                # k-chain shrinks the chunk-start proj burst from 3.4us to
                # 1.7us, so exp issue (1:1 on the span) resumes sooner.
                kdue = max(due, uidx[(pr, ic, 4 * ic)] - 1)
                # k before q when both are due together (chunk (0,0)): the k
                # weight cols are first in the DMA arrival order
                queue.append((kdue, lambda pr=pr, ic=ic: proj_qk_chunk(pr, 1, ic)))
                queue.append((due, lambda pr=pr, ic=ic: proj_qk_chunk(pr, 0, ic)))
            for t in range(TT):
                # PV of unit u is emitted during iteration u+1 (after that
                # iteration's drain), so V tiles are due one unit late --
                # keeps V chains off the first-exp critical path. First-need
                # is the earliest unit (any pair) whose j-tiles cover t.
                due_v = (
                    min(
                        uidx[(pr, ic, t)]
                        for pr in range(PAIRS)
                        for ic in range(TCH)
                        if 4 * ic + 4 > t
                    )
                    + 1
                )
                queue.append((due_v, lambda t=t: proj_v(t)))
            queue.sort(key=lambda kv: kv[0])

            qi = [0]

            def drain_due(i, lookahead=0, limit=None):
                n = 0
                while qi[0] < len(queue) and queue[qi[0]][0] <= i + lookahead:
                    if lookahead and limit is not None and n >= limit:
                        break
                    queue[qi[0]][1]()
                    qi[0] += 1
                    n += 1

            # S(u) strictly before PV(u-1): exp(u) must issue as early as
            # possible -- ScalarE is lockstep-coupled and any delay in exp
            # issue extends the span ~1:1 (PV-first orderings measured
            # consistently worse).
            last_chunk = chunks[-1]
            pend = []
            for i, (pr, ic, g) in enumerate(units):
                drain_due(i)
                pend.append(emit_s_exp(pr, ic, g))
                if len(pend) > 1:
                    u = pend.pop(0)
                    emit_pv(u)
                    maybe_epilogue(u, (u[0], u[1]) == last_chunk)
                drain_due(i, lookahead=4, limit=1)
            for u in pend:
                emit_pv(u)
                maybe_epilogue(u, (u[0], u[1]) == last_chunk)
            drain_due(len(units))

    nc.compile()
    return nc


def _get_nc():
    if "nc" not in _CACHE:
        _CACHE["nc"] = _build_nc()
    return _CACHE["nc"]


def _fold(arr):
    """[DIM, cols] -> [128, CC, cols] partition-folded wave layout."""
    return np.ascontiguousarray(
        arr.reshape(CC, 128, arr.shape[1]).transpose(1, 0, 2)
    ).astype(BF16)


def _make_in_maps(x, w_kqv, b_kqv):
    """Shard: core c -> batch c//2, head-group c%2 (heads hg*8..hg*8+7)."""
    in_maps = []
    for c in range(N_CORES):
        b, hg = divmod(c, 2)
        h0 = hg * HPC
        xT = x[b].T

        # weight columns, all transposed to [DIM(c), out]:
        # [k p0 | q p0 | k p1 | q p1 | ... | v heads (512)]
        # (k rows live at 0+, q rows at 1024+ in the torch kqv weight)
        cols = []
        for p in range(PAIRS):
            for which in (1, 0):
                base = (1 - which) * DIM  # which==1 -> k rows, 0 -> q rows
                rows = w_kqv[base + (h0 + 2 * p) * D : base + (h0 + 2 * p + 2) * D, :]
                cols.append(rows.T)
        cols.append(w_kqv[2 * DIM + h0 * D : 2 * DIM + (h0 + HPC) * D, :].T)
        wT = np.concatenate(cols, axis=1)

        bqk = np.empty((128, 2 * PAIRS), np.float32)
        for p in range(PAIRS):
            bqk[:, p] = b_kqv[DIM + (h0 + 2 * p) * D : DIM + (h0 + 2 * p + 2) * D]
            bqk[:, PAIRS + p] = b_kqv[(h0 + 2 * p) * D : (h0 + 2 * p + 2) * D]
        bv = np.tile(
            b_kqv[2 * DIM + h0 * D : 2 * DIM + (h0 + HPC) * D][None, :].astype(
                np.float32
            ),
            (128, 1),
        )
        in_maps.append(
            {
                "xa": _fold(xT[:, 0:512]),
                "xb": _fold(xT[:, 512:1024]),
                "xc": _fold(xT[:, 1024:2048]),
                "w1k": _fold(wT[:, 0:128]),
                "w1q": _fold(wT[:, 128:256]),
                "w2": _fold(wT[:, 256:512]),
                "w3": _fold(wT[:, 512:1024]),
                "wv": _fold(wT[:, 1024:1536]),
                "bqk": bqk,
                "bv": bv,
            }
        )
    return in_maps


def run(x, w_kqv, b_kqv, trace=False, **kwargs):
    from concourse.bass_utils import run_bass_kernel_spmd

    nc = _get_nc()
    in_maps = _make_in_maps(x, w_kqv, b_kqv)
    res = run_bass_kernel_spmd(
        nc, in_maps, core_ids=list(range(N_CORES)), trace=trace, **kwargs
    )
    out = np.empty((BATCH, SEQ, DIM), np.float32)
    for c in range(N_CORES):
        b, hg = divmod(c, 2)
        # staged layout [p, ic, pr, a, c] -> [ic*512 + a*128 + p, pr*128 + c]
        arr = res.results[c]["out"]
        out[b, :, hg * 512 : (hg + 1) * 512] = arr.transpose(1, 3, 0, 2, 4).reshape(
            SEQ, 512
        )
    return out, res


def kernel(x, w_kqv, b_kqv):
    args = (
        np.asarray(x, np.float32),
        np.asarray(w_kqv, np.float32),
        np.asarray(b_kqv, np.float32),
    )
    try:
        out, _ = run(*args)
    except Exception:
        # transient NRT/device errors recover on retry
        out, _ = run(*args)
    return out
